# revision 1
# baseline (speedup 1.0000x reference)
"""CRF NLL loss kernel for Trainium2 (8 NeuronCores, data-parallel over batch).

Math: the forward recurrence alpha_{t} = LSE_j(alpha_{t-1,j} + trans[j,k]) + emit_t
is computed in probability space:  P_t = Eemit_t * (Etrans^T @ P_{t-1})
with P_t = exp(alpha_t - D_t), Eemit_t = exp(emit_t - d_t), Etrans = exp(trans),
and per-step normalizers d_t = mean_b LSE_k(emit[t,b,:]) (host-precomputed) that
keep P in f32 range. Device work per step is one PE matmul + one DVE multiply.
Mask handling: run unmasked, ship P_t for t >= TOFF back to HBM; host selects
t = L_b - 1 per sequence and finishes logZ_b = log(w . P) + D_{L_b-1}.
The gold-path score is pure gather work, done on host in f64.
"""

import numpy as np

import concourse.bacc as bacc
import concourse.mybir as mybir
import concourse.tile as tile
from concourse.bass_utils import run_bass_kernel_spmd

T, B, N = 512, 256, 128
NCORES = 8
BL = B // NCORES          # 32 sequences per core
TOFF = 255                # earliest t we may need (L_b-1 >= T//2 - 1 = 255)
NOUT = T - TOFF           # 257 shipped P tiles
CHUNK = 32                # emit steps per DMA chunk

LAST_RESULTS = None       # BassKernelResults of the last run (for profiling)

_compiled = {}


def _build_nc():
    nc = bacc.Bacc("TRN2", target_bir_lowering=False, debug=False,
                   num_devices=NCORES)
    f32 = mybir.dt.float32
    eemit = nc.dram_tensor("eemit", [N, T * BL], f32, kind="ExternalInput")
    etr = nc.dram_tensor("etr", [N, N], f32, kind="ExternalInput")
    p0 = nc.dram_tensor("p0", [N, BL], f32, kind="ExternalInput")
    pout = nc.dram_tensor("pout", [N, NOUT * BL], f32, kind="ExternalOutput")

    with tile.TileContext(nc) as tc:
        with (
            tc.tile_pool(name="const", bufs=1) as cpool,
            tc.tile_pool(name="emitc", bufs=16) as epool,
            tc.tile_pool(name="pstate", bufs=4) as ppool,
            tc.tile_pool(name="psum", bufs=3, space="PSUM") as spool,
        ):
            m_tile = cpool.tile([N, N], f32, tag="weights")
            nc.sync.dma_start(m_tile[:], etr[:])

            p_cur = ppool.tile([N, BL], f32, tag="p")
            nc.sync.dma_start(p_cur[:], p0[:])

            n_chunks = (T + CHUNK - 1) // CHUNK
            chunks = [None] * n_chunks

            def load_chunk(c):
                w = min(CHUNK, T - c * CHUNK) * BL
                t_ = epool.tile([N, CHUNK * BL], f32, tag="emit")
                nc.sync.dma_start(t_[:, :w],
                                  eemit[:, c * CHUNK * BL: c * CHUNK * BL + w])
                chunks[c] = t_

            for c_ in range(n_chunks):
                load_chunk(c_)
            for t in range(1, T):
                c, off = divmod(t, CHUNK)
                s = spool.tile([N, BL], f32, tag="s")
                nc.tensor.matmul(s[:], m_tile[:], p_cur[:],
                                 start=True, stop=True)
                p_new = ppool.tile([N, BL], f32, tag="p")
                nc.vector.tensor_tensor(
                    p_new[:], s[:],
                    chunks[c][:, off * BL:(off + 1) * BL],
                    mybir.AluOpType.mult)
                if t >= TOFF:
                    o = t - TOFF
                    nc.sync.dma_start(pout[:, o * BL:(o + 1) * BL], p_new[:])
                p_cur = p_new
    nc.compile()
    return nc


def kernel(emit, target, mask, trans, strans, etrans):
    global LAST_RESULTS
    emit = np.asarray(emit, dtype=np.float32)
    target = np.asarray(target, dtype=np.int32)
    mask = np.asarray(mask)
    trans = np.asarray(trans, dtype=np.float32)
    strans = np.asarray(strans, dtype=np.float32)
    etrans = np.asarray(etrans, dtype=np.float32)

    # --- host preprocessing ---
    # per-step normalizer d_t (f64): mean over batch of LSE_k emit[t]
    e64 = emit.astype(np.float64)
    m_t = e64.max(axis=2, keepdims=True)
    lse = (m_t[..., 0] + np.log(np.exp(e64 - m_t).sum(axis=2)))  # [T,B]
    d = lse.mean(axis=1)                                         # [T]
    d[0] = 0.0
    D = np.cumsum(d)                                             # [T]

    # Eemit[t,b,k] = exp(emit - d_t), laid out [k, t*BL+b] per core
    eem = np.exp(e64 - d[:, None, None]).astype(np.float32)      # [T,B,N]
    eem[0] = 0.0
    # P0 = exp(strans + emit[0])  -> [N, B]
    p0_full = np.exp(strans[None, :].astype(np.float64) + e64[0]).astype(
        np.float32).T                                            # [N,B]
    etr = np.exp(trans.astype(np.float64)).astype(np.float32)    # [N,N] (j,k)

    in_maps = []
    for c in range(NCORES):
        sl = slice(c * BL, (c + 1) * BL)
        # [T,BL,N] -> [N,T,BL] -> [N, T*BL]
        ee = np.ascontiguousarray(
            eem[:, sl, :].transpose(2, 0, 1).reshape(N, T * BL))
        in_maps.append({
            "eemit": ee,
            "etr": etr,
            "p0": np.ascontiguousarray(p0_full[:, sl]),
        })

    if "nc" not in _compiled:
        _compiled["nc"] = _build_nc()
    nc = _compiled["nc"]

    res = run_bass_kernel_spmd(nc, in_maps, core_ids=list(range(NCORES)))
    LAST_RESULTS = res

    # --- host postprocessing ---
    L = mask.astype(np.int64).sum(axis=0)                        # [B]
    ends = L - 1
    w = np.exp(etrans.astype(np.float64))                        # [N]
    logZ = 0.0
    for c in range(NCORES):
        pout = res.results[c]["pout"].astype(np.float64)         # [N, NOUT*BL]
        for bl in range(BL):
            b = c * BL + bl
            t_end = int(ends[b])
            p_vec = pout[:, (t_end - TOFF) * BL + bl]
            logZ += np.log((w * p_vec).sum()) + D[t_end]

    # gold score (f64, mirrors reference)
    tb = np.arange(B)
    emit_sc = np.take_along_axis(e64, target[:, :, None].astype(np.int64),
                                 axis=2)[..., 0]                 # [T,B]
    trans_sc = trans.astype(np.float64)[target[:-1], target[1:]]  # [T-1,B]
    scores = emit_sc.copy()
    scores[1:] += trans_sc
    score = np.where(mask, scores, 0.0).sum()
    score += strans.astype(np.float64)[target[0]].sum()
    score += etrans.astype(np.float64)[target[ends, tb]].sum()

    loss = (logZ - score) / B
    return np.float32(loss)



# revision 3
# speedup vs baseline: 2.1587x; 2.1587x over previous
"""CRF NLL loss kernel for Trainium2 (8 NeuronCores, data-parallel over batch).

Math: the forward recurrence alpha_t = LSE_j(alpha_{t-1,j} + trans[j,k]) + emit_t
is computed in probability space:  P_t = Eemit_t * (Etrans^T @ P_{t-1})
with per-step normalizers d_t = mean_b LSE_k(emit[t,b,:]) (host-precomputed)
keeping P in f32 range.

Meet-in-the-middle: a forward chain produces P_255 (255 serial steps) while an
independent backward chain runs from t=511 down to t=256 producing
X_256 = Ehat_256 * (M @ X_257), so logZ_b = log(P_255 . (M @ X_256)) + D[end_b].
Variable sequence lengths are handled exactly by rewriting the backward
emission columns on the host: beyond a sequence's end the state is held at the
Perron vector r of M (each padded step scales by 1/lambda), and the single
boundary step uses v/r with v = M^{-1} w (w = exp(etrans)) which maps r -> w.
This halves the serial-latency-bound span (the dominant cost) vs a single
forward sweep.  The gold-path score is pure gather work, done on host in f64.
"""

import numpy as np

import concourse.bacc as bacc
import concourse.mybir as mybir
import concourse.tile as tile
from concourse.bass_utils import run_bass_kernel_spmd

T, B, N = 512, 256, 128
NCORES = 8
BL = B // NCORES          # 32 sequences per core
NSTEP = 255               # serial steps per chain (fwd: t=1..255, bwd: t=510..256)
CHUNK = 32                # emit steps per DMA chunk
ELT_ENGINE = "vector"     # "gpsimd" (Pool) or "vector" (DVE)

LAST_RESULTS = None       # BassKernelResults of the last run (for profiling)

_compiled = {}


def _build_nc():
    nc = bacc.Bacc("TRN2", target_bir_lowering=False, debug=False,
                   num_devices=NCORES)
    f32 = mybir.dt.float32
    efwd = nc.dram_tensor("efwd", [N, NSTEP * BL], f32, kind="ExternalInput")
    ebwd = nc.dram_tensor("ebwd", [N, NSTEP * BL], f32, kind="ExternalInput")
    etr = nc.dram_tensor("etr", [N, N], f32, kind="ExternalInput")
    etrT = nc.dram_tensor("etrT", [N, N], f32, kind="ExternalInput")
    p0 = nc.dram_tensor("p0", [N, BL], f32, kind="ExternalInput")
    x0 = nc.dram_tensor("x0", [N, BL], f32, kind="ExternalInput")
    pout = nc.dram_tensor("pout", [N, BL], f32, kind="ExternalOutput")
    xout = nc.dram_tensor("xout", [N, BL], f32, kind="ExternalOutput")

    elt = getattr(nc, ELT_ENGINE).tensor_tensor

    with tile.TileContext(nc) as tc:
        with (
            tc.tile_pool(name="const", bufs=1) as cpool,
            tc.tile_pool(name="emitf", bufs=8) as efpool,
            tc.tile_pool(name="emitb", bufs=8) as ebpool,
            tc.tile_pool(name="pstate", bufs=4) as fppool,
            tc.tile_pool(name="xstate", bufs=4) as bppool,
            tc.tile_pool(name="psumf", bufs=3, space="PSUM") as fspool,
            tc.tile_pool(name="psumb", bufs=3, space="PSUM") as bspool,
        ):
            mF = cpool.tile([N, N], f32, tag="wf")
            nc.sync.dma_start(mF[:], etr[:])
            mB = cpool.tile([N, N], f32, tag="wb")
            nc.sync.dma_start(mB[:], etrT[:])

            p_cur = fppool.tile([N, BL], f32, tag="p")
            nc.sync.dma_start(p_cur[:], p0[:])
            x_cur = bppool.tile([N, BL], f32, tag="x")
            nc.sync.dma_start(x_cur[:], x0[:])

            n_chunks = (NSTEP + CHUNK - 1) // CHUNK
            fch = [None] * n_chunks
            bch = [None] * n_chunks
            for c in range(n_chunks):
                w = min(CHUNK, NSTEP - c * CHUNK) * BL
                tF = efpool.tile([N, CHUNK * BL], f32, tag="ef")
                nc.sync.dma_start(tF[:, :w],
                                  efwd[:, c * CHUNK * BL: c * CHUNK * BL + w])
                fch[c] = tF
                tB = ebpool.tile([N, CHUNK * BL], f32, tag="eb")
                nc.sync.dma_start(tB[:, :w],
                                  ebwd[:, c * CHUNK * BL: c * CHUNK * BL + w])
                bch[c] = tB

            for s in range(NSTEP):
                c, off = divmod(s, CHUNK)
                sl = slice(off * BL, (off + 1) * BL)

                sF = fspool.tile([N, BL], f32, tag="sf")
                nc.tensor.matmul(sF[:], mF[:], p_cur[:], start=True, stop=True)
                p_new = fppool.tile([N, BL], f32, tag="p")
                elt(p_new[:], sF[:], fch[c][:, sl], mybir.AluOpType.mult)
                p_cur = p_new

                sB = bspool.tile([N, BL], f32, tag="sb")
                nc.tensor.matmul(sB[:], mB[:], x_cur[:], start=True, stop=True)
                x_new = bppool.tile([N, BL], f32, tag="x")
                elt(x_new[:], sB[:], bch[c][:, sl], mybir.AluOpType.mult)
                x_cur = x_new

            nc.sync.dma_start(pout[:], p_cur[:])
            nc.sync.dma_start(xout[:], x_cur[:])
    nc.compile()
    return nc


def kernel(emit, target, mask, trans, strans, etrans):
    global LAST_RESULTS
    emit = np.asarray(emit, dtype=np.float32)
    target = np.asarray(target, dtype=np.int32)
    mask = np.asarray(mask)
    trans = np.asarray(trans, dtype=np.float32)
    strans = np.asarray(strans, dtype=np.float32)
    etrans = np.asarray(etrans, dtype=np.float32)

    # --- host preprocessing ---
    # per-step normalizer d_t (f64): mean over batch of LSE_k emit[t]
    e64 = emit.astype(np.float64)
    m_t = e64.max(axis=2, keepdims=True)
    lse = (m_t[..., 0] + np.log(np.exp(e64 - m_t).sum(axis=2)))  # [T,B]
    d = lse.mean(axis=1)                                         # [T]
    d[0] = 0.0
    D = np.cumsum(d)                                             # [T]

    eemn = np.exp(e64 - d[:, None, None])                        # [T,B,N]
    M64 = np.exp(trans.astype(np.float64))                       # [N,N] (j,k)
    w64 = np.exp(etrans.astype(np.float64))                      # [N]

    # Perron vector/value of M64 and v = M^{-1} w for the backward padding
    r = np.ones(N, dtype=np.float64)
    for _ in range(60):
        r = M64 @ r
        r /= r.mean()
    lam = float((r @ (M64 @ r)) / (r @ r))
    v = np.linalg.solve(M64, w64)                                # [N]

    L = mask.astype(np.int64).sum(axis=0)                        # [B]
    ends = L - 1                                                 # in [255, 511]

    # forward: P0 and E'_t for t=1..255, laid out [N, t, B]
    P0 = np.exp(strans[None, :].astype(np.float64) + e64[0]).T   # [N,B]
    EF = np.ascontiguousarray(eemn[1:NSTEP + 1].transpose(2, 0, 1))  # [N,255,B]

    # backward emissions Ehat_t for t=256..510 (indexed i=t-256) + init X_511
    tt = np.arange(256, 511)                                     # [255]
    EB = eemn[256:511].transpose(2, 0, 1).copy()                 # [N,255,B]
    pad = tt[None, :] > L[:, None]                               # [B,255] t > L_b
    bnd = tt[None, :] == L[:, None]                              # [B,255] t == L_b
    padT = pad.T[None, :, :]                                     # [1,255,B]
    bndT = bnd.T[None, :, :]
    EB = np.where(padT, 1.0 / lam, EB)
    EB = np.where(bndT, (v / r)[:, None, None], EB)
    # consumption order: step s consumes Ehat_{510-s}  (i = 254-s)
    EBc = np.ascontiguousarray(EB[:, ::-1, :])                   # [N,255,B]

    X0 = np.empty((N, B), dtype=np.float64)                      # X_511 per col
    full = L == T                                                # L_b = 512
    last = L == T - 1                                            # L_b = 511
    rest = ~(full | last)
    if full.any():
        X0[:, full] = (eemn[511, full, :] * w64[None, :]).T
    if last.any():
        X0[:, last] = v[:, None]
    if rest.any():
        X0[:, rest] = (r / lam)[:, None]

    in_maps = []
    for c in range(NCORES):
        sl = slice(c * BL, (c + 1) * BL)
        in_maps.append({
            "efwd": np.ascontiguousarray(
                EF[:, :, sl].reshape(N, NSTEP * BL)).astype(np.float32),
            "ebwd": np.ascontiguousarray(
                EBc[:, :, sl].reshape(N, NSTEP * BL)).astype(np.float32),
            "etr": M64.astype(np.float32),
            "etrT": np.ascontiguousarray(M64.T).astype(np.float32),
            "p0": np.ascontiguousarray(P0[:, sl]).astype(np.float32),
            "x0": np.ascontiguousarray(X0[:, sl]).astype(np.float32),
        })

    if "nc" not in _compiled:
        _compiled["nc"] = _build_nc()
    nc = _compiled["nc"]

    res = run_bass_kernel_spmd(nc, in_maps, core_ids=list(range(NCORES)))
    LAST_RESULTS = res

    # --- host postprocessing ---
    P255 = np.concatenate(
        [res.results[c]["pout"].astype(np.float64) for c in range(NCORES)],
        axis=1)                                                  # [N,B]
    X256 = np.concatenate(
        [res.results[c]["xout"].astype(np.float64) for c in range(NCORES)],
        axis=1)                                                  # [N,B]
    Q255 = M64 @ X256                                            # [N,B]
    dot = (P255 * Q255).sum(axis=0)                              # [B]
    logZ = (np.log(dot) + D[ends]).sum()

    # gold score (f64, mirrors reference)
    tb = np.arange(B)
    emit_sc = np.take_along_axis(e64, target[:, :, None].astype(np.int64),
                                 axis=2)[..., 0]                 # [T,B]
    trans_sc = trans.astype(np.float64)[target[:-1], target[1:]]  # [T-1,B]
    scores = emit_sc.copy()
    scores[1:] += trans_sc
    score = np.where(mask, scores, 0.0).sum()
    score += strans.astype(np.float64)[target[0]].sum()
    score += etrans.astype(np.float64)[target[ends, tb]].sum()

    loss = (logZ - score) / B
    return np.float32(loss)


# revision 8
# speedup vs baseline: 2.2077x; 1.0227x over previous
"""CRF NLL loss kernel for Trainium2 (8 NeuronCores, data-parallel over batch).

Math: the forward recurrence alpha_t = LSE_j(alpha_{t-1,j} + trans[j,k]) + emit_t
is computed in probability space:  P_t = Eemit_t * (Etrans^T @ P_{t-1})
with per-step normalizers d_t = mean_b LSE_k(emit[t,b,:]) (host-precomputed)
keeping P in f32 range.

Meet-in-the-middle: a forward chain produces P_255 (255 serial steps) while an
independent backward chain runs from t=511 down to t=256 producing
X_256 = Ehat_256 * (M @ X_257), so logZ_b = log(P_255 . (M @ X_256)) + D[end_b].
Variable sequence lengths are handled exactly by rewriting the backward
emission columns on the host: beyond a sequence's end the state is held at the
Perron vector r of M (each padded step scales by 1/lambda), and the single
boundary step uses v/r with v = M^{-1} w (w = exp(etrans)) which maps r -> w.
This halves the serial-latency-bound span (the dominant cost) vs a single
forward sweep.  The gold-path score is pure gather work, done on host in f64.
"""

import numpy as np

import concourse.bacc as bacc
import concourse.mybir as mybir
import concourse.tile as tile
from concourse.bass_utils import run_bass_kernel_spmd

T, B, N = 512, 256, 128
NCORES = 8
BL = B // NCORES          # 32 sequences per core
NSTEP = 255               # serial steps per chain (fwd: t=1..255, bwd: t=510..256)
CHUNK = 32                # emit steps per DMA chunk
CHUNK0 = 8                # first chunk is small so the chains start early
ELT_ENGINE = "vector"     # "gpsimd" (Pool) or "vector" (DVE)

LAST_RESULTS = None       # BassKernelResults of the last run (for profiling)

_compiled = {}


def _build_nc():
    nc = bacc.Bacc("TRN2", target_bir_lowering=False, debug=False,
                   num_devices=NCORES)
    f32 = mybir.dt.float32
    efwd = nc.dram_tensor("efwd", [N, NSTEP * BL], f32, kind="ExternalInput")
    ebwd = nc.dram_tensor("ebwd", [N, NSTEP * BL], f32, kind="ExternalInput")
    etr = nc.dram_tensor("etr", [N, N], f32, kind="ExternalInput")
    etrT = nc.dram_tensor("etrT", [N, N], f32, kind="ExternalInput")
    p0 = nc.dram_tensor("p0", [N, BL], f32, kind="ExternalInput")
    x0 = nc.dram_tensor("x0", [N, BL], f32, kind="ExternalInput")
    pqout = nc.dram_tensor("pqout", [N, 2 * BL], f32, kind="ExternalOutput")

    elt = getattr(nc, ELT_ENGINE).tensor_tensor

    # step -> (chunk index, offset) with a small first chunk so the chains
    # can start while the bulk of the emissions is still in flight
    bounds = [0, CHUNK0]
    while bounds[-1] < NSTEP:
        bounds.append(min(bounds[-1] + CHUNK, NSTEP))
    n_chunks = len(bounds) - 1

    with tile.TileContext(nc) as tc:
        with (
            tc.tile_pool(name="const", bufs=1) as cpool,
            tc.tile_pool(name="emitf", bufs=n_chunks) as efpool,
            tc.tile_pool(name="emitb", bufs=n_chunks) as ebpool,
            tc.tile_pool(name="pstate", bufs=4) as fppool,
            tc.tile_pool(name="xstate", bufs=4) as bppool,
            tc.tile_pool(name="psumf", bufs=3, space="PSUM") as fspool,
            tc.tile_pool(name="psumb", bufs=3, space="PSUM") as bspool,
            tc.tile_pool(name="outp", bufs=1) as opool,
        ):
            # Critical lead-in loads spread across 4 DMA queues so their
            # issue/generation latencies overlap.
            p_cur = fppool.tile([N, BL], f32, tag="p")
            nc.sync.dma_start(p_cur[:], p0[:])
            x_cur = bppool.tile([N, BL], f32, tag="x")
            nc.scalar.dma_start(x_cur[:], x0[:])
            mF = cpool.tile([N, N], f32, tag="wf")
            nc.sync.dma_start(mF[:], etr[:])
            mB = cpool.tile([N, N], f32, tag="wb")
            nc.scalar.dma_start(mB[:], etrT[:])

            fch = [None] * n_chunks
            bch = [None] * n_chunks

            def load_chunk(c, f_eng, b_eng):
                lo, hi = bounds[c] * BL, bounds[c + 1] * BL
                tF = efpool.tile([N, CHUNK * BL], f32, tag="ef")
                f_eng.dma_start(tF[:, :hi - lo], efwd[:, lo:hi])
                fch[c] = tF
                tB = ebpool.tile([N, CHUNK * BL], f32, tag="eb")
                b_eng.dma_start(tB[:, :hi - lo], ebwd[:, lo:hi])
                bch[c] = tB

            load_chunk(0, nc.gpsimd, nc.gpsimd)
            for c in range(1, n_chunks):
                load_chunk(c, nc.sync, nc.scalar)

            out_t = opool.tile([N, 2 * BL], f32, tag="pq")

            for s in range(NSTEP):
                c = next(i for i in range(n_chunks) if bounds[i + 1] > s)
                off = s - bounds[c]
                sl = slice(off * BL, (off + 1) * BL)
                last = s == NSTEP - 1

                sF = fspool.tile([N, BL], f32, tag="sf")
                nc.tensor.matmul(sF[:], mF[:], p_cur[:], start=True, stop=True)
                p_new = out_t if last else fppool.tile([N, BL], f32, tag="p")
                elt(p_new[:, 0:BL] if last else p_new[:],
                    sF[:], fch[c][:, sl], mybir.AluOpType.mult)
                p_cur = p_new

                sB = bspool.tile([N, BL], f32, tag="sb")
                nc.tensor.matmul(sB[:], mB[:], x_cur[:], start=True, stop=True)
                x_new = out_t if last else bppool.tile([N, BL], f32, tag="x")
                elt(x_new[:, BL:2 * BL] if last else x_new[:],
                    sB[:], bch[c][:, sl], mybir.AluOpType.mult)
                x_cur = x_new

            nc.sync.dma_start(pqout[:], out_t[:])
    nc.compile()
    return nc


def kernel(emit, target, mask, trans, strans, etrans):
    global LAST_RESULTS
    emit = np.asarray(emit, dtype=np.float32)
    target = np.asarray(target, dtype=np.int32)
    mask = np.asarray(mask)
    trans = np.asarray(trans, dtype=np.float32)
    strans = np.asarray(strans, dtype=np.float32)
    etrans = np.asarray(etrans, dtype=np.float32)

    # --- host preprocessing ---
    # per-step normalizer d_t (f64): mean over batch of LSE_k emit[t]
    e64 = emit.astype(np.float64)
    m_t = e64.max(axis=2, keepdims=True)
    lse = (m_t[..., 0] + np.log(np.exp(e64 - m_t).sum(axis=2)))  # [T,B]
    d = lse.mean(axis=1)                                         # [T]
    d[0] = 0.0
    D = np.cumsum(d)                                             # [T]

    eemn = np.exp(e64 - d[:, None, None])                        # [T,B,N]
    M64 = np.exp(trans.astype(np.float64))                       # [N,N] (j,k)
    w64 = np.exp(etrans.astype(np.float64))                      # [N]

    # Perron vector/value of M64 and v = M^{-1} w for the backward padding
    r = np.ones(N, dtype=np.float64)
    for _ in range(60):
        r = M64 @ r
        r /= r.mean()
    lam = float((r @ (M64 @ r)) / (r @ r))
    v = np.linalg.solve(M64, w64)                                # [N]

    L = mask.astype(np.int64).sum(axis=0)                        # [B]
    ends = L - 1                                                 # in [255, 511]

    # forward: P0 and E'_t for t=1..255, laid out [N, t, B]
    P0 = np.exp(strans[None, :].astype(np.float64) + e64[0]).T   # [N,B]
    EF = np.ascontiguousarray(eemn[1:NSTEP + 1].transpose(2, 0, 1))  # [N,255,B]

    # backward emissions Ehat_t for t=256..510 (indexed i=t-256) + init X_511
    tt = np.arange(256, 511)                                     # [255]
    EB = eemn[256:511].transpose(2, 0, 1).copy()                 # [N,255,B]
    pad = tt[None, :] > L[:, None]                               # [B,255] t > L_b
    bnd = tt[None, :] == L[:, None]                              # [B,255] t == L_b
    padT = pad.T[None, :, :]                                     # [1,255,B]
    bndT = bnd.T[None, :, :]
    EB = np.where(padT, 1.0 / lam, EB)
    EB = np.where(bndT, (v / r)[:, None, None], EB)
    # consumption order: step s consumes Ehat_{510-s}  (i = 254-s)
    EBc = np.ascontiguousarray(EB[:, ::-1, :])                   # [N,255,B]

    X0 = np.empty((N, B), dtype=np.float64)                      # X_511 per col
    full = L == T                                                # L_b = 512
    last = L == T - 1                                            # L_b = 511
    rest = ~(full | last)
    if full.any():
        X0[:, full] = (eemn[511, full, :] * w64[None, :]).T
    if last.any():
        X0[:, last] = v[:, None]
    if rest.any():
        X0[:, rest] = (r / lam)[:, None]

    in_maps = []
    for c in range(NCORES):
        sl = slice(c * BL, (c + 1) * BL)
        in_maps.append({
            "efwd": np.ascontiguousarray(
                EF[:, :, sl].reshape(N, NSTEP * BL)).astype(np.float32),
            "ebwd": np.ascontiguousarray(
                EBc[:, :, sl].reshape(N, NSTEP * BL)).astype(np.float32),
            "etr": M64.astype(np.float32),
            "etrT": np.ascontiguousarray(M64.T).astype(np.float32),
            "p0": np.ascontiguousarray(P0[:, sl]).astype(np.float32),
            "x0": np.ascontiguousarray(X0[:, sl]).astype(np.float32),
        })

    if "nc" not in _compiled:
        _compiled["nc"] = _build_nc()
    nc = _compiled["nc"]

    res = run_bass_kernel_spmd(nc, in_maps, core_ids=list(range(NCORES)))
    LAST_RESULTS = res

    # --- host postprocessing ---
    P255 = np.concatenate(
        [res.results[c]["pqout"][:, :BL].astype(np.float64)
         for c in range(NCORES)], axis=1)                        # [N,B]
    X256 = np.concatenate(
        [res.results[c]["pqout"][:, BL:].astype(np.float64)
         for c in range(NCORES)], axis=1)                        # [N,B]
    Q255 = M64 @ X256                                            # [N,B]
    dot = (P255 * Q255).sum(axis=0)                              # [B]
    logZ = (np.log(dot) + D[ends]).sum()

    # gold score (f64, mirrors reference)
    tb = np.arange(B)
    emit_sc = np.take_along_axis(e64, target[:, :, None].astype(np.int64),
                                 axis=2)[..., 0]                 # [T,B]
    trans_sc = trans.astype(np.float64)[target[:-1], target[1:]]  # [T-1,B]
    scores = emit_sc.copy()
    scores[1:] += trans_sc
    score = np.where(mask, scores, 0.0).sum()
    score += strans.astype(np.float64)[target[0]].sum()
    score += etrans.astype(np.float64)[target[ends, tb]].sum()

    loss = (logZ - score) / B
    return np.float32(loss)


# revision 13
# speedup vs baseline: 2.2206x; 1.0058x over previous
"""CRF NLL loss kernel for Trainium2 (8 NeuronCores, data-parallel over batch).

Math: the forward recurrence alpha_t = LSE_j(alpha_{t-1,j} + trans[j,k]) + emit_t
is computed in probability space:  P_t = Eemit_t * (Etrans^T @ P_{t-1})
with per-step normalizers d_t = mean_b LSE_k(emit[t,b,:]) (host-precomputed)
keeping P in f32 range.

Meet-in-the-middle: a forward chain produces P_255 (255 serial steps) while an
independent backward chain runs from t=511 down to t=256 producing
X_256 = Ehat_256 * (M @ X_257), so logZ_b = log(P_255 . (M @ X_256)) + D[end_b].
Variable sequence lengths are handled exactly by rewriting the backward
emission columns on the host: beyond a sequence's end the state is held at the
Perron vector r of M (each padded step scales by 1/lambda), and the single
boundary step uses v/r with v = M^{-1} w (w = exp(etrans)) which maps r -> w.
This halves the serial-latency-bound span (the dominant cost) vs a single
forward sweep.  The gold-path score is pure gather work, done on host in f64.
"""

import numpy as np

import concourse.bacc as bacc
import concourse.mybir as mybir
import concourse.tile as tile
from concourse.bass_utils import run_bass_kernel_spmd

T, B, N = 512, 256, 128
NCORES = 8
BL = B // NCORES          # 32 sequences per core
NSTEP = 255               # serial steps per chain (fwd: t=1..255, bwd: t=510..256)
CHUNK = 32                # emit steps per DMA chunk
CHUNK0 = 8                # first chunk is small so the chains start early
ELT_ENGINE = "vector"     # "gpsimd" (Pool) or "vector" (DVE)

LAST_RESULTS = None       # BassKernelResults of the last run (for profiling)

_compiled = {}


def _build_nc():
    nc = bacc.Bacc("TRN2", target_bir_lowering=False, debug=False,
                   num_devices=NCORES)
    f32 = mybir.dt.float32
    # initf/initb pack [state0 | weights | first emission chunk] so each
    # chain's critical lead-in needs a single HWDGE generation slot (the
    # descriptor generator is shared across queues and serializes at ~630ns).
    IW = BL + N + CHUNK0 * BL
    initf = nc.dram_tensor("initf", [N, IW], f32, kind="ExternalInput")
    initb = nc.dram_tensor("initb", [N, IW], f32, kind="ExternalInput")
    efwd = nc.dram_tensor("efwd", [N, (NSTEP - CHUNK0) * BL], f32,
                          kind="ExternalInput")
    ebwd = nc.dram_tensor("ebwd", [N, (NSTEP - CHUNK0) * BL], f32,
                          kind="ExternalInput")
    pqout = nc.dram_tensor("pqout", [N, 2 * BL], f32, kind="ExternalOutput")

    elt = getattr(nc, ELT_ENGINE).tensor_tensor

    # step -> (chunk index, offset) with a small first chunk so the chains
    # can start while the bulk of the emissions is still in flight
    bounds = [0, CHUNK0]
    while bounds[-1] < NSTEP:
        bounds.append(min(bounds[-1] + CHUNK, NSTEP))
    n_chunks = len(bounds) - 1

    with tile.TileContext(nc) as tc:
        with (
            tc.tile_pool(name="const", bufs=1) as cpool,
            tc.tile_pool(name="emitf", bufs=n_chunks) as efpool,
            tc.tile_pool(name="emitb", bufs=n_chunks) as ebpool,
            tc.tile_pool(name="pstate", bufs=4) as fppool,
            tc.tile_pool(name="xstate", bufs=4) as bppool,
            tc.tile_pool(name="psumf", bufs=3, space="PSUM") as fspool,
            tc.tile_pool(name="psumb", bufs=3, space="PSUM") as bspool,
            tc.tile_pool(name="outp", bufs=1) as opool,
        ):
            # One merged critical DMA per chain on separate queues.
            IW = BL + N + CHUNK0 * BL
            tF0 = cpool.tile([N, IW], f32, tag="initf")
            nc.sync.dma_start(tF0[:], initf[:])
            tB0 = cpool.tile([N, IW], f32, tag="initb")
            nc.scalar.dma_start(tB0[:], initb[:])
            p_cur = tF0[:, 0:BL]
            mF = tF0[:, BL:BL + N]
            x_cur = tB0[:, 0:BL]
            mB = tB0[:, BL:BL + N]

            fch = [tF0] * n_chunks
            bch = [tB0] * n_chunks
            fbase = [0] * n_chunks
            fbase[0] = BL + N

            for c in range(1, n_chunks):
                lo = (bounds[c] - CHUNK0) * BL
                hi = (bounds[c + 1] - CHUNK0) * BL
                tF = efpool.tile([N, CHUNK * BL], f32, tag="ef")
                nc.sync.dma_start(tF[:, :hi - lo], efwd[:, lo:hi])
                fch[c] = tF
                tB = ebpool.tile([N, CHUNK * BL], f32, tag="eb")
                nc.scalar.dma_start(tB[:, :hi - lo], ebwd[:, lo:hi])
                bch[c] = tB

            out_t = opool.tile([N, 2 * BL], f32, tag="pq")

            for s in range(NSTEP):
                c = next(i for i in range(n_chunks) if bounds[i + 1] > s)
                off = s - bounds[c]
                sl = slice(fbase[c] + off * BL, fbase[c] + (off + 1) * BL)
                last = s == NSTEP - 1

                sF = fspool.tile([N, BL], f32, tag="sf")
                nc.tensor.matmul(sF[:], mF, p_cur, start=True, stop=True)
                if last:
                    p_dst = out_t[:, 0:BL]
                else:
                    p_new = fppool.tile([N, BL], f32, tag="p")
                    p_dst = p_new[:]
                elt(p_dst, sF[:], fch[c][:, sl], mybir.AluOpType.mult)
                p_cur = p_dst

                sB = bspool.tile([N, BL], f32, tag="sb")
                nc.tensor.matmul(sB[:], mB, x_cur, start=True, stop=True)
                if last:
                    x_dst = out_t[:, BL:2 * BL]
                else:
                    x_new = bppool.tile([N, BL], f32, tag="x")
                    x_dst = x_new[:]
                elt(x_dst, sB[:], bch[c][:, sl], mybir.AluOpType.mult)
                x_cur = x_dst

            nc.sync.dma_start(pqout[:], out_t[:])
    nc.compile()
    return nc


def kernel(emit, target, mask, trans, strans, etrans):
    global LAST_RESULTS
    emit = np.asarray(emit, dtype=np.float32)
    target = np.asarray(target, dtype=np.int32)
    mask = np.asarray(mask)
    trans = np.asarray(trans, dtype=np.float32)
    strans = np.asarray(strans, dtype=np.float32)
    etrans = np.asarray(etrans, dtype=np.float32)

    # --- host preprocessing ---
    # per-step normalizer d_t (f64): mean over batch of LSE_k emit[t]
    e64 = emit.astype(np.float64)
    m_t = e64.max(axis=2, keepdims=True)
    lse = (m_t[..., 0] + np.log(np.exp(e64 - m_t).sum(axis=2)))  # [T,B]
    d = lse.mean(axis=1)                                         # [T]
    d[0] = 0.0
    D = np.cumsum(d)                                             # [T]

    eemn = np.exp(e64 - d[:, None, None])                        # [T,B,N]
    M64 = np.exp(trans.astype(np.float64))                       # [N,N] (j,k)
    w64 = np.exp(etrans.astype(np.float64))                      # [N]

    # Perron vector/value of M64 and v = M^{-1} w for the backward padding
    r = np.ones(N, dtype=np.float64)
    for _ in range(60):
        r = M64 @ r
        r /= r.mean()
    lam = float((r @ (M64 @ r)) / (r @ r))
    v = np.linalg.solve(M64, w64)                                # [N]

    L = mask.astype(np.int64).sum(axis=0)                        # [B]
    ends = L - 1                                                 # in [255, 511]

    # forward: P0 and E'_t for t=1..255, laid out [N, t, B]
    P0 = np.exp(strans[None, :].astype(np.float64) + e64[0]).T   # [N,B]
    EF = np.ascontiguousarray(eemn[1:NSTEP + 1].transpose(2, 0, 1))  # [N,255,B]

    # backward emissions Ehat_t for t=256..510 (indexed i=t-256) + init X_511
    tt = np.arange(256, 511)                                     # [255]
    EB = eemn[256:511].transpose(2, 0, 1).copy()                 # [N,255,B]
    pad = tt[None, :] > L[:, None]                               # [B,255] t > L_b
    bnd = tt[None, :] == L[:, None]                              # [B,255] t == L_b
    padT = pad.T[None, :, :]                                     # [1,255,B]
    bndT = bnd.T[None, :, :]
    EB = np.where(padT, 1.0 / lam, EB)
    EB = np.where(bndT, (v / r)[:, None, None], EB)
    # consumption order: step s consumes Ehat_{510-s}  (i = 254-s)
    EBc = np.ascontiguousarray(EB[:, ::-1, :])                   # [N,255,B]

    X0 = np.empty((N, B), dtype=np.float64)                      # X_511 per col
    full = L == T                                                # L_b = 512
    last = L == T - 1                                            # L_b = 511
    rest = ~(full | last)
    if full.any():
        X0[:, full] = (eemn[511, full, :] * w64[None, :]).T
    if last.any():
        X0[:, last] = v[:, None]
    if rest.any():
        X0[:, rest] = (r / lam)[:, None]

    M32 = M64.astype(np.float32)
    MT32 = np.ascontiguousarray(M64.T).astype(np.float32)
    in_maps = []
    for c in range(NCORES):
        sl = slice(c * BL, (c + 1) * BL)
        ef = EF[:, :, sl].reshape(N, NSTEP * BL).astype(np.float32)
        eb = EBc[:, :, sl].reshape(N, NSTEP * BL).astype(np.float32)
        in_maps.append({
            "initf": np.ascontiguousarray(np.concatenate(
                [P0[:, sl].astype(np.float32), M32, ef[:, :CHUNK0 * BL]],
                axis=1)),
            "initb": np.ascontiguousarray(np.concatenate(
                [X0[:, sl].astype(np.float32), MT32, eb[:, :CHUNK0 * BL]],
                axis=1)),
            "efwd": np.ascontiguousarray(ef[:, CHUNK0 * BL:]),
            "ebwd": np.ascontiguousarray(eb[:, CHUNK0 * BL:]),
        })

    if "nc" not in _compiled:
        _compiled["nc"] = _build_nc()
    nc = _compiled["nc"]

    res = run_bass_kernel_spmd(nc, in_maps, core_ids=list(range(NCORES)))
    LAST_RESULTS = res

    # --- host postprocessing ---
    P255 = np.concatenate(
        [res.results[c]["pqout"][:, :BL].astype(np.float64)
         for c in range(NCORES)], axis=1)                        # [N,B]
    X256 = np.concatenate(
        [res.results[c]["pqout"][:, BL:].astype(np.float64)
         for c in range(NCORES)], axis=1)                        # [N,B]
    Q255 = M64 @ X256                                            # [N,B]
    dot = (P255 * Q255).sum(axis=0)                              # [B]
    logZ = (np.log(dot) + D[ends]).sum()

    # gold score (f64, mirrors reference)
    tb = np.arange(B)
    emit_sc = np.take_along_axis(e64, target[:, :, None].astype(np.int64),
                                 axis=2)[..., 0]                 # [T,B]
    trans_sc = trans.astype(np.float64)[target[:-1], target[1:]]  # [T-1,B]
    scores = emit_sc.copy()
    scores[1:] += trans_sc
    score = np.where(mask, scores, 0.0).sum()
    score += strans.astype(np.float64)[target[0]].sum()
    score += etrans.astype(np.float64)[target[ends, tb]].sum()

    loss = (logZ - score) / B
    return np.float32(loss)


# revision 14
# speedup vs baseline: 2.2232x; 1.0012x over previous
"""CRF NLL loss kernel for Trainium2 (8 NeuronCores, data-parallel over batch).

Math: the forward recurrence alpha_t = LSE_j(alpha_{t-1,j} + trans[j,k]) + emit_t
is computed in probability space:  P_t = Eemit_t * (Etrans^T @ P_{t-1})
with per-step normalizers d_t = mean_b LSE_k(emit[t,b,:]) (host-precomputed)
keeping P in f32 range.

Meet-in-the-middle: a forward chain produces P_255 (255 serial steps) while an
independent backward chain runs from t=511 down to t=256 producing
X_256 = Ehat_256 * (M @ X_257), so logZ_b = log(P_255 . (M @ X_256)) + D[end_b].
Variable sequence lengths are handled exactly by rewriting the backward
emission columns on the host: beyond a sequence's end the state is held at the
Perron vector r of M (each padded step scales by 1/lambda), and the single
boundary step uses v/r with v = M^{-1} w (w = exp(etrans)) which maps r -> w.
This halves the serial-latency-bound span (the dominant cost) vs a single
forward sweep.  The gold-path score is pure gather work, done on host in f64.
"""

import numpy as np

import concourse.bacc as bacc
import concourse.mybir as mybir
import concourse.tile as tile
from concourse.bass_utils import run_bass_kernel_spmd

T, B, N = 512, 256, 128
NCORES = 8
BL = B // NCORES          # 32 sequences per core
NSTEP = 255               # serial steps per chain (fwd: t=1..255, bwd: t=510..256)
CHUNK = 32                # emit steps per DMA chunk
CHUNK0 = 8                # first chunk is small so the chains start early
ELT_ENGINE = "vector"     # "gpsimd" (Pool) or "vector" (DVE)

LAST_RESULTS = None       # BassKernelResults of the last run (for profiling)

_compiled = {}


def _build_nc():
    nc = bacc.Bacc("TRN2", target_bir_lowering=False, debug=False,
                   num_devices=NCORES)
    f32 = mybir.dt.float32
    # initf/initb pack [state0 | weights | first emission chunk] so each
    # chain's critical lead-in needs a single HWDGE generation slot (the
    # descriptor generator is shared across queues and serializes at ~630ns).
    IW = BL + N + CHUNK0 * BL
    initf = nc.dram_tensor("initf", [N, IW], f32, kind="ExternalInput")
    initb = nc.dram_tensor("initb", [N, IW], f32, kind="ExternalInput")
    efwd = nc.dram_tensor("efwd", [N, (NSTEP - CHUNK0) * BL], f32,
                          kind="ExternalInput")
    ebwd = nc.dram_tensor("ebwd", [N, (NSTEP - CHUNK0) * BL], f32,
                          kind="ExternalInput")
    pqout = nc.dram_tensor("pqout", [N, 2 * BL], f32, kind="ExternalOutput")

    elt = getattr(nc, ELT_ENGINE).tensor_tensor

    # step -> (chunk index, offset) with a small first chunk so the chains
    # can start while the bulk of the emissions is still in flight
    bounds = [0, CHUNK0]
    while bounds[-1] < NSTEP:
        bounds.append(min(bounds[-1] + CHUNK, NSTEP))
    n_chunks = len(bounds) - 1

    with tile.TileContext(nc) as tc:
        with (
            tc.tile_pool(name="const", bufs=1) as cpool,
            tc.tile_pool(name="emitf", bufs=n_chunks) as efpool,
            tc.tile_pool(name="emitb", bufs=n_chunks) as ebpool,
            tc.tile_pool(name="pstate", bufs=4) as fppool,
            tc.tile_pool(name="xstate", bufs=4) as bppool,
            tc.tile_pool(name="psumf", bufs=3, space="PSUM") as fspool,
            tc.tile_pool(name="psumb", bufs=3, space="PSUM") as bspool,
            tc.tile_pool(name="outp", bufs=1) as opool,
        ):
            # One merged critical DMA per chain on separate queues.
            IW = BL + N + CHUNK0 * BL
            tF0 = cpool.tile([N, IW], f32, tag="initf")
            nc.sync.dma_start(tF0[:], initf[:])
            tB0 = cpool.tile([N, IW], f32, tag="initb")
            nc.gpsimd.dma_start(tB0[:], initb[:])
            p_cur = tF0[:, 0:BL]
            mF = tF0[:, BL:BL + N]
            x_cur = tB0[:, 0:BL]
            mB = tB0[:, BL:BL + N]

            fch = [tF0] * n_chunks
            bch = [tB0] * n_chunks
            fbase = [0] * n_chunks
            fbase[0] = BL + N

            for c in range(1, n_chunks):
                lo = (bounds[c] - CHUNK0) * BL
                hi = (bounds[c + 1] - CHUNK0) * BL
                tF = efpool.tile([N, CHUNK * BL], f32, tag="ef")
                nc.sync.dma_start(tF[:, :hi - lo], efwd[:, lo:hi])
                fch[c] = tF
                tB = ebpool.tile([N, CHUNK * BL], f32, tag="eb")
                nc.scalar.dma_start(tB[:, :hi - lo], ebwd[:, lo:hi])
                bch[c] = tB

            out_t = opool.tile([N, 2 * BL], f32, tag="pq")

            for s in range(NSTEP):
                c = next(i for i in range(n_chunks) if bounds[i + 1] > s)
                off = s - bounds[c]
                sl = slice(fbase[c] + off * BL, fbase[c] + (off + 1) * BL)
                last = s == NSTEP - 1

                sF = fspool.tile([N, BL], f32, tag="sf")
                nc.tensor.matmul(sF[:], mF, p_cur, start=True, stop=True)
                if last:
                    p_dst = out_t[:, 0:BL]
                else:
                    p_new = fppool.tile([N, BL], f32, tag="p")
                    p_dst = p_new[:]
                elt(p_dst, sF[:], fch[c][:, sl], mybir.AluOpType.mult)
                p_cur = p_dst

                sB = bspool.tile([N, BL], f32, tag="sb")
                nc.tensor.matmul(sB[:], mB, x_cur, start=True, stop=True)
                if last:
                    x_dst = out_t[:, BL:2 * BL]
                else:
                    x_new = bppool.tile([N, BL], f32, tag="x")
                    x_dst = x_new[:]
                elt(x_dst, sB[:], bch[c][:, sl], mybir.AluOpType.mult)
                x_cur = x_dst

            nc.sync.dma_start(pqout[:], out_t[:])
    nc.compile()
    return nc


def kernel(emit, target, mask, trans, strans, etrans):
    global LAST_RESULTS
    emit = np.asarray(emit, dtype=np.float32)
    target = np.asarray(target, dtype=np.int32)
    mask = np.asarray(mask)
    trans = np.asarray(trans, dtype=np.float32)
    strans = np.asarray(strans, dtype=np.float32)
    etrans = np.asarray(etrans, dtype=np.float32)

    # --- host preprocessing ---
    # per-step normalizer d_t (f64): mean over batch of LSE_k emit[t]
    e64 = emit.astype(np.float64)
    m_t = e64.max(axis=2, keepdims=True)
    lse = (m_t[..., 0] + np.log(np.exp(e64 - m_t).sum(axis=2)))  # [T,B]
    d = lse.mean(axis=1)                                         # [T]
    d[0] = 0.0
    D = np.cumsum(d)                                             # [T]

    eemn = np.exp(e64 - d[:, None, None])                        # [T,B,N]
    M64 = np.exp(trans.astype(np.float64))                       # [N,N] (j,k)
    w64 = np.exp(etrans.astype(np.float64))                      # [N]

    # Perron vector/value of M64 and v = M^{-1} w for the backward padding
    r = np.ones(N, dtype=np.float64)
    for _ in range(60):
        r = M64 @ r
        r /= r.mean()
    lam = float((r @ (M64 @ r)) / (r @ r))
    v = np.linalg.solve(M64, w64)                                # [N]

    L = mask.astype(np.int64).sum(axis=0)                        # [B]
    ends = L - 1                                                 # in [255, 511]

    # forward: P0 and E'_t for t=1..255, laid out [N, t, B]
    P0 = np.exp(strans[None, :].astype(np.float64) + e64[0]).T   # [N,B]
    EF = np.ascontiguousarray(eemn[1:NSTEP + 1].transpose(2, 0, 1))  # [N,255,B]

    # backward emissions Ehat_t for t=256..510 (indexed i=t-256) + init X_511
    tt = np.arange(256, 511)                                     # [255]
    EB = eemn[256:511].transpose(2, 0, 1).copy()                 # [N,255,B]
    pad = tt[None, :] > L[:, None]                               # [B,255] t > L_b
    bnd = tt[None, :] == L[:, None]                              # [B,255] t == L_b
    padT = pad.T[None, :, :]                                     # [1,255,B]
    bndT = bnd.T[None, :, :]
    EB = np.where(padT, 1.0 / lam, EB)
    EB = np.where(bndT, (v / r)[:, None, None], EB)
    # consumption order: step s consumes Ehat_{510-s}  (i = 254-s)
    EBc = np.ascontiguousarray(EB[:, ::-1, :])                   # [N,255,B]

    X0 = np.empty((N, B), dtype=np.float64)                      # X_511 per col
    full = L == T                                                # L_b = 512
    last = L == T - 1                                            # L_b = 511
    rest = ~(full | last)
    if full.any():
        X0[:, full] = (eemn[511, full, :] * w64[None, :]).T
    if last.any():
        X0[:, last] = v[:, None]
    if rest.any():
        X0[:, rest] = (r / lam)[:, None]

    M32 = M64.astype(np.float32)
    MT32 = np.ascontiguousarray(M64.T).astype(np.float32)
    in_maps = []
    for c in range(NCORES):
        sl = slice(c * BL, (c + 1) * BL)
        ef = EF[:, :, sl].reshape(N, NSTEP * BL).astype(np.float32)
        eb = EBc[:, :, sl].reshape(N, NSTEP * BL).astype(np.float32)
        in_maps.append({
            "initf": np.ascontiguousarray(np.concatenate(
                [P0[:, sl].astype(np.float32), M32, ef[:, :CHUNK0 * BL]],
                axis=1)),
            "initb": np.ascontiguousarray(np.concatenate(
                [X0[:, sl].astype(np.float32), MT32, eb[:, :CHUNK0 * BL]],
                axis=1)),
            "efwd": np.ascontiguousarray(ef[:, CHUNK0 * BL:]),
            "ebwd": np.ascontiguousarray(eb[:, CHUNK0 * BL:]),
        })

    if "nc" not in _compiled:
        _compiled["nc"] = _build_nc()
    nc = _compiled["nc"]

    res = run_bass_kernel_spmd(nc, in_maps, core_ids=list(range(NCORES)))
    LAST_RESULTS = res

    # --- host postprocessing ---
    P255 = np.concatenate(
        [res.results[c]["pqout"][:, :BL].astype(np.float64)
         for c in range(NCORES)], axis=1)                        # [N,B]
    X256 = np.concatenate(
        [res.results[c]["pqout"][:, BL:].astype(np.float64)
         for c in range(NCORES)], axis=1)                        # [N,B]
    Q255 = M64 @ X256                                            # [N,B]
    dot = (P255 * Q255).sum(axis=0)                              # [B]
    logZ = (np.log(dot) + D[ends]).sum()

    # gold score (f64, mirrors reference)
    tb = np.arange(B)
    emit_sc = np.take_along_axis(e64, target[:, :, None].astype(np.int64),
                                 axis=2)[..., 0]                 # [T,B]
    trans_sc = trans.astype(np.float64)[target[:-1], target[1:]]  # [T-1,B]
    scores = emit_sc.copy()
    scores[1:] += trans_sc
    score = np.where(mask, scores, 0.0).sum()
    score += strans.astype(np.float64)[target[0]].sum()
    score += etrans.astype(np.float64)[target[ends, tb]].sum()

    loss = (logZ - score) / B
    return np.float32(loss)
